# revision 45
# baseline (speedup 1.0000x reference)
"""Grouped Conv1d (B=4, T=512, G=129, F=96 -> O=96, K=3, pad=1) on 8 trn2 cores.

Sharding: 129 groups = 16 full groups per core + group 128 split across all
8 cores by (batch b = core//2, T-half = core%2).  SPMD: every core runs the
identical program on its own slice.

PE strategy: the F*K=288-row contraction per (group, batch) is split into
nine 32-row chunks.  Groups are processed 4 at a time, each pinned to its
own 32-partition row range (group g' at partitions 32g'): four chunk
matmuls on four different PE row groups stream CONCURRENTLY through the
array (tile_position row tiling; measured ~216ns per 4xN=512 round at full
clock).  The four streams run the identical (a, h, j, c) schedule shifted
by one round each (stagger): PSUM banks then close one per round instead
of four at once, so the two drain engines (DVE/ACT, the only PSUM readers)
never fall a burst behind and accumulator recycling never stalls the PE.
x is float8e3 (e3m4), w fp16 (fp8 weights slow LDWEIGHTS 114->140ns and
cost +43ns per 216ns round -- not worth the DMA savings), accumulate fp32,
drain to fp16.

DMA strategy: x/w live in a wrap layout spanning all 128 partitions so
every big transfer is balanced across the SDMA engines.  Traffic uses
three queues: sync (Q1, HWDGE) carries the x0 gate piece then half the
stores; scalar (Q10, HWDGE, issued by ACT prologue-only since ACT is
drain-critical) carries weights + most x; gpsimd (Q0, SWDGE, ~280B/ns
with ~5us issue-to-data lag, idle engine) carries x3 and the other half
of the stores.  The tail group's tensors are FOLDED into spare columns
of xm[0]/wt so they need no separate small transfers (sub-1KB-packet
DMAs run at 20-60B/ns), and its matmuls run after the main stream where
the PE is idle anyway.  Warm-up matmuls on zeroed scratch bridge the PE
from the engine preamble to the first x arrival: the PE p-state needs
~3us of continuous work, and the HAM full-clock grant is a hard budget
(k=8 for N*3413ns, N~10-12, from sustained-activity onset, then k=4
regardless), so the body must start early, stay dense, and end soon —
work past the window, including the fixed ~4.4us NEFF epilogue of
per-engine semaphore clears, runs at half clock.
"""

from contextlib import ExitStack

import numpy as np
import ml_dtypes

import concourse.bass as bass
import concourse.mybir as mybir
import concourse.tile as tile
from concourse import bacc
from concourse.bass_utils import run_bass_kernel_spmd

B, T, G, F, O, K = 4, 512, 129, 96, 96, 3
NCORES = 8
GPC = 16          # full groups per core
NB = 4            # x/out DMA + compute batches (4 groups each)
GPB = 4           # groups per batch
TP = T + 2        # padded T
TE = T // 2       # tail-group T chunk per core
TEP = TE + 2
SLOT = B * TP     # x free-dim elems per (group, f-strip): 4*514
XES = 3 * SLOT    # xe (tail-group x) column offset inside xm
XCOLS = XES + TEP
WES = NB * 3 * K * O   # wte (tail-group w) column offset inside wt
WCOLS = WES + K * O
NWARM = 7         # PE warm-up matmuls: bridge engine preamble to first x arrival
SPL = 292         # drain split: DVE cols [0:SPL) vs ACT [SPL:T) (1.44 vs 1.9 ns/col)


def build_program():
    nc = bacc.Bacc("TRN2", target_bir_lowering=False, debug=False,
                   num_devices=NCORES)

    f32 = mybir.dt.float32
    f16 = mybir.dt.float16
    f8 = mybir.dt.float8e3

    xm = nc.dram_tensor("xm", [NB, 128, XCOLS], f8, kind="ExternalInput")
    wt = nc.dram_tensor("wt", [128, WCOLS], f16, kind="ExternalInput")
    om = nc.dram_tensor("om", [NB, O, GPB * B * T], f16, kind="ExternalOutput")
    oe = nc.dram_tensor("oe", [O, TE], f16, kind="ExternalOutput")

    with ExitStack() as ctx:
        tc = ctx.enter_context(tile.TileContext(nc))
        wpool = ctx.enter_context(tc.tile_pool(name="w", bufs=1))
        xpool = ctx.enter_context(tc.tile_pool(name="x", bufs=4))
        opool = ctx.enter_context(tc.tile_pool(name="o", bufs=4))
        pspool = ctx.enter_context(tc.tile_pool(name="ps", bufs=8, space="PSUM"))

        w_sb = wpool.tile([128, WCOLS], f16)
        xdum = wpool.tile([128, 256], f8)
        wdum = wpool.tile([128, O], f16)

        # memsets on gpsimd (the earliest-ready engine) so warm-up matmuls
        # start the moment the PE preamble ends.
        nc.gpsimd.memset(wdum[:], 0.0)
        nc.gpsimd.memset(xdum[:], 0.0)
        psdum = pspool.tile([O, 256], f32, tag="ps", name="psdum")
        for i in range(NWARM):
            nc.tensor.matmul(psdum[:], wdum[:], xdum[:],
                             start=(i == 0), stop=(i == NWARM - 1))

        kw = 3 * K * O
        x_tiles = {}
        for a in range(NB):
            x_tiles[a] = xpool.tile([128, XCOLS], f8, tag="x", name=f"x{a}")
        HB = 6 * TP                                    # columns per b-half
        HJ = 3 * TP                                    # columns per (hb, j)
        # Q1 (sync) carries ONLY the x0 gate piece and then the stores: the
        # 6.3MB of stores alone saturate one ring for ~29us, so all other
        # input traffic rides Q10 (scalar, gate-ordered) and x3 trickles on
        # gpsimd SWDGE (needed last, issued once, keeps ACT's issue count
        # low enough that its first drains aren't pushed past PSUM slack).
        nc.sync.dma_start(x_tiles[0][:, :HJ], xm[0][:, :HJ])
        # a=0's weights in two chunk-aligned pieces: [0:480] covers chunks
        # c0-c4 and lands ~0.6us before the full 864 cols would; the rest
        # streams right behind it, well before c5 is needed (+1.1us).
        nc.scalar.dma_start(w_sb[:, :5 * O], wt[:, :5 * O])
        nc.scalar.dma_start(w_sb[:, 5 * O:kw], wt[:, 5 * O:kw])
        # tiny dummy ACTIVATE: pulls ACT's 1.3us activation-table load into
        # the prologue instead of delaying the first real PSUM drain
        adum = wpool.tile([O, 1], f16)
        nc.scalar.add(adum[:], wdum[:O, :1], 0.0)
        nc.gpsimd.dma_start(x_tiles[3][:, :XES], xm[3][:, :XES])
        nc.scalar.dma_start(x_tiles[0][:, HJ:HB], xm[0][:, HJ:HB])
        nc.scalar.dma_start(x_tiles[0][:, HB:], xm[0][:, HB:])  # incl. xe
        nc.scalar.dma_start(x_tiles[1][:, :HJ], xm[1][:, :HJ])
        nc.scalar.dma_start(w_sb[:, kw:2 * kw], wt[:, kw:2 * kw])
        nc.scalar.dma_start(x_tiles[1][:, HJ:HB], xm[1][:, HJ:HB])
        nc.scalar.dma_start(x_tiles[1][:, HB:XES], xm[1][:, HB:XES])
        nc.scalar.dma_start(w_sb[:, 2 * kw:], wt[:, 2 * kw:])   # incl. wte
        nc.scalar.dma_start(x_tiles[2][:, :HJ], xm[2][:, :HJ])
        nc.scalar.dma_start(x_tiles[2][:, HJ:XES], xm[2][:, HJ:XES])

        # Staggered main loop: stream gl runs the whole (a, h, j, c)
        # schedule shifted by gl rounds.
        sched = [(a, h, j, c) for a in range(NB) for h in range(2)
                 for j in range(2) for c in range(9)]
        NR = len(sched)
        pss = {}
        o_tiles = {}
        for r in range(NR + 3):
            for gl in range(4):
                rp = r - gl
                if not 0 <= rp < NR:
                    continue
                a, h, j, c = sched[rp]
                fs, kk = c // 3, c % 3
                if c == 0:
                    pss[gl] = pspool.tile(
                        [O, T], f32, tag="ps", name=f"ps{a}{h}{gl}{j}")
                nc.tensor.matmul(
                    pss[gl][:],
                    w_sb[32 * gl:32 * gl + 32,
                         ((a * 3 + fs) * K + kk) * O:
                         ((a * 3 + fs) * K + kk + 1) * O],
                    x_tiles[a][32 * gl:32 * gl + 32,
                               ((h * 2 + j) * 3 + fs) * TP + kk:
                               ((h * 2 + j) * 3 + fs) * TP + kk + T],
                    start=(c == 0), stop=(c == 8),
                    tile_position=(32 * gl, 0),
                )
                if c == 8:
                    # this lane's (a, h, j) accumulator just closed: drain
                    # (plain fp32->fp16 copies, bias is added host-side)
                    # split DVE/ACT, then store the quad once gl3 drains.
                    if (a, h) not in o_tiles:
                        o_tiles[(a, h)] = opool.tile(
                            [O, GPB * 2 * T], f16, tag="o", name=f"o{a}_{h}")
                    o_half = o_tiles[(a, h)]
                    final = (a == NB - 1 and h == 1 and j == 1)
                    c0 = (j * GPB + gl) * T
                    nc.vector.tensor_copy(
                        o_half[:, c0:c0 + SPL], pss[gl][:, :SPL])
                    nc.scalar.add(
                        o_half[:, c0 + SPL:c0 + T], pss[gl][:, SPL:], 0.0)
                    if final:
                        # last quad: store per group as it drains, split
                        # across both rings so the four 0.6us issues run
                        # in parallel instead of serializing on sync
                        q0 = (h * 2 + j) * GPB * T
                        eng = nc.sync if gl % 2 == 0 else nc.scalar
                        eng.dma_start(
                            om[a][:, q0 + gl * T:q0 + (gl + 1) * T],
                            o_half[:, (j * GPB + gl) * T:
                                   (j * GPB + gl + 1) * T])
                    elif gl == 3:
                        # alternate quad stores between the sync ring (Q1)
                        # and gpsimd SWDGE (Q0, ~280B/ns, idle engine --
                        # one ring alone backs up ~4us of stores; never the
                        # drain engines mid-stream).  Q0 data lags its
                        # issue ~5us, so it also takes a3h0's two quads
                        # (the lag is absorbed before the end) keeping the
                        # sync ring clear for the last quad + finals.
                        q = a * 4 + h * 2 + j
                        if q == 14:
                            # last mid-stream quad rides the scalar ring,
                            # idle since the prologue: ACT issues it in the
                            # natural gap between its j0 and j1 drain
                            # bursts, freeing sync for the final per-group
                            # pieces (stagger slack absorbs any drain skew)
                            eng = nc.scalar
                        else:
                            eng = (nc.gpsimd if q == 13 or q % 2 == 0
                                   else nc.sync)
                        eng.dma_start(
                            om[a][:, (h * 2 + j) * GPB * T:
                                  (h * 2 + j + 1) * GPB * T],
                            o_half[:, j * GPB * T:(j + 1) * GPB * T])

        # tail group (g=128): runs after the main stream where the PE is
        # idle anyway; its tensors rode along inside xm[0]/wt.
        pstail = pspool.tile([O, TE], f32, tag="ps", name="pstail")
        for kk in range(K):
            nc.tensor.matmul(
                pstail[:],
                w_sb[:F, WES + kk * O:WES + (kk + 1) * O],
                x_tiles[0][:F, XES + kk:XES + kk + TE],
                start=(kk == 0), stop=(kk == K - 1),
            )
        oe_sb = wpool.tile([O, TE], f16)
        nc.vector.tensor_copy(oe_sb[:, :150], pstail[:, :150])
        nc.scalar.add(oe_sb[:, 150:], pstail[:, 150:], 0.0)
        nc.sync.dma_start(oe[:], oe_sb[:])

    # NOTE: a trailing "keep-alive" DMA to hold the clock through the NEFF
    # epilogue does NOT work: the HAM full-clock grant is a hard budget
    # (k=8 for exactly N*3413ns from sustained-activity onset, N~10-12,
    # then k=4 regardless of DMA activity), and the epilogue's per-engine
    # queue DRAIN waits for any in-flight DMA, extending the critical path.

    nc.finalize()
    return nc


def shard_inputs(x, weight, bias):
    x = np.ascontiguousarray(x, dtype=np.float32)
    weight = np.ascontiguousarray(weight, dtype=np.float32)

    xp = np.pad(x, ((0, 0), (1, 1), (0, 0), (0, 0)))          # [B, TP, G, F]
    xt = np.ascontiguousarray(xp.transpose(2, 3, 0, 1)).astype(
        ml_dtypes.float8_e3m4)                                # [G, F, B, TP]
    wtr = weight.astype(np.float16)                           # [G, O, F, K]

    in_maps = []
    for c in range(NCORES):
        g0 = c * GPC
        b_c, t0 = c // 2, (c % 2) * TE
        # x: group g'=g%4 of batch a=g//4 at partitions 32g'; free dim
        # is [b-half][b%2][f-strip][t] so each j-phase's columns are one
        # contiguous DMA range.  Tail-group x rides cols [XES:] of xm[0].
        xc = xt[g0:g0 + GPC].reshape(NB, GPB, 3, 32, 2, 2, TP)
        xc = xc.transpose(0, 1, 3, 4, 5, 2, 6)              # a,g',i,hb,j,fs,t
        xm_c = np.zeros((NB, 128, XCOLS), dtype=ml_dtypes.float8_e3m4)
        xm_c[:, :, :XES] = xc.reshape(NB, 128, XES)
        xm_c[0, :F, XES:] = xt[G - 1, :, b_c, t0:t0 + TEP]
        # w: same partition mapping; cols = (a, fs, k, o); wte at [WES:]
        wc = wtr[g0:g0 + GPC].transpose(0, 2, 3, 1)           # [16, F, K, O]
        wc = wc.reshape(NB, GPB, 3, 32, K, O)                 # a,g',fs,i,k,o
        wc = wc.transpose(1, 3, 0, 2, 4, 5)                   # g',i,a,fs,k,o
        wt_c = np.zeros((128, WCOLS), dtype=np.float16)
        wt_c[:, :WES] = wc.reshape(128, WES)
        wt_c[:F, WES:] = wtr[G - 1].transpose(1, 2, 0).reshape(F, K * O)
        in_maps.append({"xm": xm_c, "wt": wt_c})
    return in_maps


def unshard_outputs(results):
    out = np.empty((B, T, G, O), dtype=np.float32)
    for c in range(NCORES):
        om = results[c]["om"].astype(np.float32)       # [NB, O, GPB*B*T]
        # om cols = (h, j, g', T) with b = 2h + j; bias is added host-side
        om = om.reshape(NB, O, 2, 2, GPB, T)           # a,o,h,j,g',t
        om = om.transpose(2, 3, 5, 0, 4, 1)            # h,j,t,a,g',o
        out[:, :, c * GPC:(c + 1) * GPC, :] = om.reshape(B, T, GPC, O)
        b_c, t0 = c // 2, (c % 2) * TE
        out[b_c, t0:t0 + TE, G - 1, :] = results[c]["oe"].astype(np.float32).T
    return out


def run(x, weight, bias, **run_kwargs):
    nc = build_program()
    in_maps = shard_inputs(x, weight, bias)
    res = run_bass_kernel_spmd(nc, in_maps, list(range(NCORES)), **run_kwargs)
    out = unshard_outputs(res.results)
    out += np.asarray(bias, dtype=np.float32)[None, None, :, :]
    return out, res


def kernel(x, weight, bias):
    out, _ = run(x, weight, bias)
    return out


# revision 46
# speedup vs baseline: 1.0083x; 1.0083x over previous
"""Grouped Conv1d (B=4, T=512, G=129, F=96 -> O=96, K=3, pad=1) on 8 trn2 cores.

Sharding: 129 groups = 16 full groups per core + group 128 split across all
8 cores by (batch b = core//2, T-half = core%2).  SPMD: every core runs the
identical program on its own slice.

PE strategy: the F*K=288-row contraction per (group, batch) is split into
nine 32-row chunks.  Groups are processed 4 at a time, each pinned to its
own 32-partition row range (group g' at partitions 32g'): four chunk
matmuls on four different PE row groups stream CONCURRENTLY through the
array (tile_position row tiling; measured ~216ns per 4xN=512 round at full
clock).  The four streams run the identical (a, h, j, c) schedule shifted
by one round each (stagger): PSUM banks then close one per round instead
of four at once, so the two drain engines (DVE/ACT, the only PSUM readers)
never fall a burst behind and accumulator recycling never stalls the PE.
x is float8e3 (e3m4), w fp16 (fp8 weights slow LDWEIGHTS 114->140ns and
cost +43ns per 216ns round -- not worth the DMA savings), accumulate fp32,
drain to fp16.

DMA strategy: x/w live in a wrap layout spanning all 128 partitions so
every big transfer is balanced across the SDMA engines.  Traffic uses
three queues: sync (Q1, HWDGE) carries the x0 gate piece then half the
stores; scalar (Q10, HWDGE, issued by ACT prologue-only since ACT is
drain-critical) carries weights + most x; gpsimd (Q0, SWDGE, ~280B/ns
with ~5us issue-to-data lag, idle engine) carries x3 and the other half
of the stores.  The tail group's tensors are FOLDED into spare columns
of xm[0]/wt so they need no separate small transfers (sub-1KB-packet
DMAs run at 20-60B/ns), and its matmuls run after the main stream where
the PE is idle anyway.  Warm-up matmuls on zeroed scratch bridge the PE
from the engine preamble to the first x arrival: the PE p-state needs
~3us of continuous work, and the HAM full-clock grant is a hard budget
(k=8 for N*3413ns, N~10-12, from sustained-activity onset, then k=4
regardless), so the body must start early, stay dense, and end soon —
work past the window, including the fixed ~4.4us NEFF epilogue of
per-engine semaphore clears, runs at half clock.
"""

from contextlib import ExitStack

import numpy as np
import ml_dtypes

import concourse.bass as bass
import concourse.mybir as mybir
import concourse.tile as tile
from concourse import bacc
from concourse.bass_utils import run_bass_kernel_spmd

B, T, G, F, O, K = 4, 512, 129, 96, 96, 3
NCORES = 8
GPC = 16          # full groups per core
NB = 4            # x/out DMA + compute batches (4 groups each)
GPB = 4           # groups per batch
TP = T + 2        # padded T
TE = T // 2       # tail-group T chunk per core
TEP = TE + 2
SLOT = B * TP     # x free-dim elems per (group, f-strip): 4*514
XES = 3 * SLOT    # xe (tail-group x) column offset inside xm
XCOLS = XES + TEP
WES = NB * 3 * K * O   # wte (tail-group w) column offset inside wt
WCOLS = WES + K * O
NWARM = 7         # PE warm-up matmuls: bridge engine preamble to first x arrival
SPL = 292         # drain split: DVE cols [0:SPL) vs ACT [SPL:T) (1.44 vs 1.9 ns/col)


def build_program():
    nc = bacc.Bacc("TRN2", target_bir_lowering=False, debug=False,
                   num_devices=NCORES)

    f32 = mybir.dt.float32
    f16 = mybir.dt.float16
    f8 = mybir.dt.float8e3

    xm = nc.dram_tensor("xm", [NB, 128, XCOLS], f8, kind="ExternalInput")
    wt = nc.dram_tensor("wt", [128, WCOLS], f16, kind="ExternalInput")
    om = nc.dram_tensor("om", [NB, O, GPB * B * T], f16, kind="ExternalOutput")
    oe = nc.dram_tensor("oe", [O, TE], f16, kind="ExternalOutput")

    with ExitStack() as ctx:
        tc = ctx.enter_context(tile.TileContext(nc))
        wpool = ctx.enter_context(tc.tile_pool(name="w", bufs=1))
        xpool = ctx.enter_context(tc.tile_pool(name="x", bufs=4))
        opool = ctx.enter_context(tc.tile_pool(name="o", bufs=4))
        pspool = ctx.enter_context(tc.tile_pool(name="ps", bufs=8, space="PSUM"))

        w_sb = wpool.tile([128, WCOLS], f16)
        xdum = wpool.tile([128, 256], f8)
        wdum = wpool.tile([128, O], f16)

        # memsets on gpsimd (the earliest-ready engine) so warm-up matmuls
        # start the moment the PE preamble ends.
        nc.gpsimd.memset(wdum[:], 0.0)
        nc.gpsimd.memset(xdum[:], 0.0)
        psdum = pspool.tile([O, 256], f32, tag="ps", name="psdum")
        for i in range(NWARM):
            nc.tensor.matmul(psdum[:], wdum[:], xdum[:],
                             start=(i == 0), stop=(i == NWARM - 1))

        kw = 3 * K * O
        x_tiles = {}
        for a in range(NB):
            x_tiles[a] = xpool.tile([128, XCOLS], f8, tag="x", name=f"x{a}")
        HB = 6 * TP                                    # columns per b-half
        HJ = 3 * TP                                    # columns per (hb, j)
        # Q1 (sync) carries ONLY the x0 gate piece and then the stores: the
        # 6.3MB of stores alone saturate one ring for ~29us, so all other
        # input traffic rides Q10 (scalar, gate-ordered) and x3 trickles on
        # gpsimd SWDGE (needed last, issued once, keeps ACT's issue count
        # low enough that its first drains aren't pushed past PSUM slack).
        nc.sync.dma_start(x_tiles[0][:, :HJ], xm[0][:, :HJ])
        # a=0's weights as ONE piece: every finer split (by fs, by column,
        # by partition-half) was tried on hardware and lost to queue-start
        # jitter -- a late second piece stalls rounds 3-8 by up to 1.9us,
        # worse than the ~0.6us a split saves when it lands on time.
        nc.scalar.dma_start(w_sb[:, :kw], wt[:, :kw])
        # tiny dummy ACTIVATE: pulls ACT's 1.3us activation-table load into
        # the prologue instead of delaying the first real PSUM drain
        adum = wpool.tile([O, 1], f16)
        nc.scalar.add(adum[:], wdum[:O, :1], 0.0)
        nc.gpsimd.dma_start(x_tiles[3][:, :XES], xm[3][:, :XES])
        nc.scalar.dma_start(x_tiles[0][:, HJ:HB], xm[0][:, HJ:HB])
        nc.scalar.dma_start(x_tiles[0][:, HB:], xm[0][:, HB:])  # incl. xe
        nc.scalar.dma_start(x_tiles[1][:, :HJ], xm[1][:, :HJ])
        nc.scalar.dma_start(w_sb[:, kw:2 * kw], wt[:, kw:2 * kw])
        nc.scalar.dma_start(x_tiles[1][:, HJ:HB], xm[1][:, HJ:HB])
        nc.scalar.dma_start(x_tiles[1][:, HB:XES], xm[1][:, HB:XES])
        nc.scalar.dma_start(w_sb[:, 2 * kw:], wt[:, 2 * kw:])   # incl. wte
        nc.scalar.dma_start(x_tiles[2][:, :HJ], xm[2][:, :HJ])
        nc.scalar.dma_start(x_tiles[2][:, HJ:XES], xm[2][:, HJ:XES])

        # Staggered main loop: stream gl runs the whole (a, h, j, c)
        # schedule shifted by gl rounds.
        sched = [(a, h, j, c) for a in range(NB) for h in range(2)
                 for j in range(2) for c in range(9)]
        NR = len(sched)
        pss = {}
        o_tiles = {}
        for r in range(NR + 3):
            for gl in range(4):
                rp = r - gl
                if not 0 <= rp < NR:
                    continue
                a, h, j, c = sched[rp]
                fs, kk = c // 3, c % 3
                if c == 0:
                    pss[gl] = pspool.tile(
                        [O, T], f32, tag="ps", name=f"ps{a}{h}{gl}{j}")
                nc.tensor.matmul(
                    pss[gl][:],
                    w_sb[32 * gl:32 * gl + 32,
                         ((a * 3 + fs) * K + kk) * O:
                         ((a * 3 + fs) * K + kk + 1) * O],
                    x_tiles[a][32 * gl:32 * gl + 32,
                               ((h * 2 + j) * 3 + fs) * TP + kk:
                               ((h * 2 + j) * 3 + fs) * TP + kk + T],
                    start=(c == 0), stop=(c == 8),
                    tile_position=(32 * gl, 0),
                )
                if c == 8:
                    # this lane's (a, h, j) accumulator just closed: drain
                    # (plain fp32->fp16 copies, bias is added host-side)
                    # split DVE/ACT, then store the quad once gl3 drains.
                    if (a, h) not in o_tiles:
                        o_tiles[(a, h)] = opool.tile(
                            [O, GPB * 2 * T], f16, tag="o", name=f"o{a}_{h}")
                    o_half = o_tiles[(a, h)]
                    final = (a == NB - 1 and h == 1 and j == 1)
                    c0 = (j * GPB + gl) * T
                    nc.vector.tensor_copy(
                        o_half[:, c0:c0 + SPL], pss[gl][:, :SPL])
                    nc.scalar.add(
                        o_half[:, c0 + SPL:c0 + T], pss[gl][:, SPL:], 0.0)
                    if final:
                        # last quad: store per group as it drains, split
                        # across both rings so the four 0.6us issues run
                        # in parallel instead of serializing on sync
                        q0 = (h * 2 + j) * GPB * T
                        eng = nc.sync if gl % 2 == 0 else nc.scalar
                        eng.dma_start(
                            om[a][:, q0 + gl * T:q0 + (gl + 1) * T],
                            o_half[:, (j * GPB + gl) * T:
                                   (j * GPB + gl + 1) * T])
                    elif gl == 3:
                        # alternate quad stores between the sync ring (Q1)
                        # and gpsimd SWDGE (Q0, ~280B/ns, idle engine --
                        # one ring alone backs up ~4us of stores; never the
                        # drain engines mid-stream).  Q0 data lags its
                        # issue ~5us, so it also takes a3h0's two quads
                        # (the lag is absorbed before the end) keeping the
                        # sync ring clear for the last quad + finals.
                        q = a * 4 + h * 2 + j
                        if q == 14:
                            # last mid-stream quad rides the scalar ring,
                            # idle since the prologue: ACT issues it in the
                            # natural gap between its j0 and j1 drain
                            # bursts, freeing sync for the final per-group
                            # pieces (stagger slack absorbs any drain skew)
                            eng = nc.scalar
                        else:
                            eng = (nc.gpsimd if q == 13 or q % 2 == 0
                                   else nc.sync)
                        eng.dma_start(
                            om[a][:, (h * 2 + j) * GPB * T:
                                  (h * 2 + j + 1) * GPB * T],
                            o_half[:, j * GPB * T:(j + 1) * GPB * T])

        # tail group (g=128): runs after the main stream where the PE is
        # idle anyway; its tensors rode along inside xm[0]/wt.
        pstail = pspool.tile([O, TE], f32, tag="ps", name="pstail")
        for kk in range(K):
            nc.tensor.matmul(
                pstail[:],
                w_sb[:F, WES + kk * O:WES + (kk + 1) * O],
                x_tiles[0][:F, XES + kk:XES + kk + TE],
                start=(kk == 0), stop=(kk == K - 1),
            )
        oe_sb = wpool.tile([O, TE], f16)
        nc.vector.tensor_copy(oe_sb[:, :150], pstail[:, :150])
        nc.scalar.add(oe_sb[:, 150:], pstail[:, 150:], 0.0)
        nc.sync.dma_start(oe[:], oe_sb[:])

    # NOTE: a trailing "keep-alive" DMA to hold the clock through the NEFF
    # epilogue does NOT work: the HAM full-clock grant is a hard budget
    # (k=8 for exactly N*3413ns from sustained-activity onset, N~10-12,
    # then k=4 regardless of DMA activity), and the epilogue's per-engine
    # queue DRAIN waits for any in-flight DMA, extending the critical path.

    nc.finalize()
    return nc


def shard_inputs(x, weight, bias):
    x = np.ascontiguousarray(x, dtype=np.float32)
    weight = np.ascontiguousarray(weight, dtype=np.float32)

    xp = np.pad(x, ((0, 0), (1, 1), (0, 0), (0, 0)))          # [B, TP, G, F]
    xt = np.ascontiguousarray(xp.transpose(2, 3, 0, 1)).astype(
        ml_dtypes.float8_e3m4)                                # [G, F, B, TP]
    wtr = weight.astype(np.float16)                           # [G, O, F, K]

    in_maps = []
    for c in range(NCORES):
        g0 = c * GPC
        b_c, t0 = c // 2, (c % 2) * TE
        # x: group g'=g%4 of batch a=g//4 at partitions 32g'; free dim
        # is [b-half][b%2][f-strip][t] so each j-phase's columns are one
        # contiguous DMA range.  Tail-group x rides cols [XES:] of xm[0].
        xc = xt[g0:g0 + GPC].reshape(NB, GPB, 3, 32, 2, 2, TP)
        xc = xc.transpose(0, 1, 3, 4, 5, 2, 6)              # a,g',i,hb,j,fs,t
        xm_c = np.zeros((NB, 128, XCOLS), dtype=ml_dtypes.float8_e3m4)
        xm_c[:, :, :XES] = xc.reshape(NB, 128, XES)
        xm_c[0, :F, XES:] = xt[G - 1, :, b_c, t0:t0 + TEP]
        # w: same partition mapping; cols = (a, fs, k, o); wte at [WES:]
        wc = wtr[g0:g0 + GPC].transpose(0, 2, 3, 1)           # [16, F, K, O]
        wc = wc.reshape(NB, GPB, 3, 32, K, O)                 # a,g',fs,i,k,o
        wc = wc.transpose(1, 3, 0, 2, 4, 5)                   # g',i,a,fs,k,o
        wt_c = np.zeros((128, WCOLS), dtype=np.float16)
        wt_c[:, :WES] = wc.reshape(128, WES)
        wt_c[:F, WES:] = wtr[G - 1].transpose(1, 2, 0).reshape(F, K * O)
        in_maps.append({"xm": xm_c, "wt": wt_c})
    return in_maps


def unshard_outputs(results):
    out = np.empty((B, T, G, O), dtype=np.float32)
    for c in range(NCORES):
        om = results[c]["om"].astype(np.float32)       # [NB, O, GPB*B*T]
        # om cols = (h, j, g', T) with b = 2h + j; bias is added host-side
        om = om.reshape(NB, O, 2, 2, GPB, T)           # a,o,h,j,g',t
        om = om.transpose(2, 3, 5, 0, 4, 1)            # h,j,t,a,g',o
        out[:, :, c * GPC:(c + 1) * GPC, :] = om.reshape(B, T, GPC, O)
        b_c, t0 = c // 2, (c % 2) * TE
        out[b_c, t0:t0 + TE, G - 1, :] = results[c]["oe"].astype(np.float32).T
    return out


def run(x, weight, bias, **run_kwargs):
    nc = build_program()
    in_maps = shard_inputs(x, weight, bias)
    res = run_bass_kernel_spmd(nc, in_maps, list(range(NCORES)), **run_kwargs)
    out = unshard_outputs(res.results)
    out += np.asarray(bias, dtype=np.float32)[None, None, :, :]
    return out, res


def kernel(x, weight, bias):
    out, _ = run(x, weight, bias)
    return out
